# revision 4
# baseline (speedup 1.0000x reference)
"""Expert-parallel MoE MLP kernel for Trainium2 (8 NeuronCores).

Problem: x[B=2,S=1024,H=1024] f32, expert_indices[B,S] int, 16 experts,
gate/up_proj[E,H,I], down_proj[E,I,H] (H=I=1024):
    out[n] = silu(x_n @ Wg[e_n]) * (x_n @ Wu[e_n]) @ Wd[e_n].T

Sharding: expert parallelism - core c owns experts {2c, 2c+1}. The host
groups tokens by expert (the "all-to-all dispatch" runs on host since the
kernel contract is full-input -> full-output), pads each expert's token
block to a 16-multiple capacity, and each core runs dense per-expert GEMMs.

All operands are bf16 (rel err ~4e-3 vs the 2e-2 gate), which halves the
mandatory weight traffic to 12 MB/core - the roofline. Perf structure:
  - weight DMAs ride the SP HWDGE queue alone, in stream order, first in
    program order; nothing compute-dependent ever enters that FIFO, so
    the stream never stalls
  - xt and out DMAs ride the Activation HWDGE queue in parallel
  - matmuls run at the real (padded-to-16) token width: bf16 is
    1 cycle/row at any width, unlike f32r which needs >=256
  - gate/up accumulate h-outer into 8 PSUM banks so matmuls chase the
    arriving weight chunks; down_proj is packed in output-column chunks
    (3 double + 2 single j-tiles) so the last weight chunk feeds only 8
    short matmuls + one 37 KB store; that last store goes on the (by
    then idle) SP queue so it doesn't serialize behind the previous out
"""

import math

import numpy as np

E = 16
H = 1024
HT = 8           # H / 128 partition tiles
N_CORES = 8
EPC = E // N_CORES   # experts per core
NS = 4           # gate/up DMA chunks per projection (0.5 MB each)
HH = HT // NS    # h-tiles per gate/up chunk
# down chunks: j-tile groups per chunk (3 doubles, then 2 singles)
DCH = [(0, 2), (2, 2), (4, 2), (6, 1), (7, 1)]

_NC_CACHE = {}


def _build_nc(pio: int):
    """One SPMD program: EPC experts, pio real (DMA'd) token columns per
    expert. pio must be a multiple of 16, <= 512 (PSUM bank = 2 KB f32)."""
    import concourse.tile as tile
    from concourse import bacc, mybir
    from concourse.bass import ts

    f32 = mybir.dt.float32
    bf16 = mybir.dt.bfloat16

    nc = bacc.Bacc("TRN2", target_bir_lowering=False, debug=False,
                   num_devices=N_CORES)
    # gate/up packed partition-major: w[e, proj, p, h, :] = Wproj[e][h*128+p, :]
    w = nc.dram_tensor("w", [EPC, 2, 128, HT, H], bf16, kind="ExternalInput")
    # down packed j-sliced: wda[e, p, q, k, jj] = Wd[e].T[k*128+p, q*256+jj],
    # wdb[e, p, q, k, jj] = Wd[e].T[k*128+p, 768 + q*128+jj]
    wda = nc.dram_tensor("wda", [EPC, 128, 3, HT, 256], bf16,
                         kind="ExternalInput")
    wdb = nc.dram_tensor("wdb", [EPC, 128, 2, HT, 128], bf16,
                         kind="ExternalInput")
    xt = nc.dram_tensor("xt", [EPC, 128, HT, pio], bf16, kind="ExternalInput")
    out = nc.dram_tensor("out", [EPC, 128, HT, pio], bf16,
                         kind="ExternalOutput")

    with tile.TileContext(nc) as tc:
        with (
            tc.tile_pool(name="wp", bufs=13 * EPC) as wp,
            tc.tile_pool(name="xp", bufs=1) as xp,
            tc.tile_pool(name="gp", bufs=2) as gp,
            tc.tile_pool(name="ip", bufs=2) as ip,
            tc.tile_pool(name="op", bufs=2) as op,
            tc.tile_pool(name="ps", bufs=8, space="PSUM") as ps,
        ):
            # the whole weight stream, enqueued first in use order; every
            # tile is a distinct buffer so the FIFO never waits on compute
            wts = {}
            for e in range(EPC):
                for proj in range(2):
                    for q in range(NS):
                        t = wp.tile([128, HH, H], bf16, tag="w",
                                    name=f"w{e}{proj}{q}")
                        nc.sync.dma_start(t[:], w[e, proj, :, ts(q, HH), :])
                        wts[e, proj, q] = t
                for q, (j0, nj) in enumerate(DCH):
                    t = wp.tile([128, HT, nj * 128], bf16, tag="w",
                                name=f"wd{e}{q}")
                    if nj == 2:
                        nc.sync.dma_start(t[:], wda[e, :, q])
                    else:
                        nc.sync.dma_start(t[:], wdb[e, :, q - 3])
                    wts[e, 2, q] = t

            # tokens for both experts, on the Act queue (parallel to weights)
            x_sb = xp.tile([128, EPC, HT, pio], bf16)
            for e in range(EPC):
                nc.scalar.dma_start(x_sb[:, e], xt[e])

            for e in range(EPC):
                g_sb = gp.tile([128, HT, pio], f32)
                i_sb = ip.tile([128, HT, pio], bf16)
                o_sb = op.tile([128, HT, pio], bf16, tag="o")
                # gate: h-outer accumulation into 8 PSUM banks
                g_ps = [ps.tile([128, pio], f32, tag="ps", name=f"g{e}{i}")
                        for i in range(HT)]
                for h in range(HT):
                    wt = wts[e, 0, h // HH]
                    for i in range(HT):
                        nc.tensor.matmul(
                            g_ps[i][:], wt[:, h % HH, ts(i, 128)],
                            x_sb[:, e, h], start=(h == 0), stop=(h == HT - 1))
                for i in range(HT):
                    nc.scalar.activation(
                        g_sb[:, i], g_ps[i][:],
                        mybir.ActivationFunctionType.Silu)
                # up
                u_ps = [ps.tile([128, pio], f32, tag="ps", name=f"u{e}{i}")
                        for i in range(HT)]
                for h in range(HT):
                    wt = wts[e, 1, h // HH]
                    for i in range(HT):
                        nc.tensor.matmul(
                            u_ps[i][:], wt[:, h % HH, ts(i, 128)],
                            x_sb[:, e, h], start=(h == 0), stop=(h == HT - 1))
                for i in range(HT):
                    nc.vector.tensor_mul(i_sb[:, i], g_sb[:, i], u_ps[i][:])
                # down: j-sliced chunks; each chunk finishes its j-tiles and
                # ships them immediately
                for q, (j0, nj) in enumerate(DCH):
                    wt = wts[e, 2, q]
                    for jl in range(nj):
                        o_ps = ps.tile([128, pio], f32, tag="ps",
                                       name=f"o{e}{q}{jl}")
                        for k in range(HT):
                            nc.tensor.matmul(
                                o_ps[:], wt[:, k, ts(jl, 128)], i_sb[:, k],
                                start=(k == 0), stop=(k == HT - 1))
                        nc.vector.tensor_copy(o_sb[:, j0 + jl], o_ps[:])
                    last = e == EPC - 1 and q == len(DCH) - 1
                    eng = nc.sync if last else nc.scalar
                    eng.dma_start(out[e, :, j0:j0 + nj, :],
                                  o_sb[:, j0:j0 + nj, :])
    nc.compile()
    return nc


def _get_nc(pio: int):
    if pio not in _NC_CACHE:
        _NC_CACHE[pio] = _build_nc(pio)
    return _NC_CACHE[pio]


_ROUND_CAP = 512          # max tokens/expert per device round (PSUM bank)


def _kernel_once(x, expert_indices, gate_proj, up_proj, down_proj):
    import ml_dtypes
    from concourse.bass_utils import run_bass_kernel_spmd

    bf16 = np.dtype(ml_dtypes.bfloat16)
    x = np.ascontiguousarray(x, dtype=np.float32)
    b, s, h = x.shape
    assert (h, gate_proj.shape) == (H, (E, H, H)), (x.shape, gate_proj.shape)

    n = b * s
    xf = x.reshape(n, h)
    idx = np.asarray(expert_indices).reshape(n).astype(np.int64)

    order = np.argsort(idx, kind="stable")       # token ids grouped by expert
    counts = np.bincount(idx, minlength=E)
    starts = np.zeros(E + 1, dtype=np.int64)
    np.cumsum(counts, out=starts[1:])
    maxc = int(counts.max())
    assert maxc <= _ROUND_CAP
    pio = max(16, 16 * math.ceil(maxc / 16))

    # per-core weight packing (bf16, partition-major)
    wr = np.stack([gate_proj, up_proj], axis=1).astype(bf16) \
        .reshape(N_CORES, EPC, 2, HT, 128, H).transpose(0, 1, 2, 4, 3, 5)
    dt = np.ascontiguousarray(down_proj.transpose(0, 2, 1)).astype(bf16)
    wda = dt[:, :, :768].reshape(N_CORES, EPC, HT, 128, 3, 256) \
        .transpose(0, 1, 3, 4, 2, 5)
    wdb = dt[:, :, 768:].reshape(N_CORES, EPC, HT, 128, 2, 128) \
        .transpose(0, 1, 3, 4, 2, 5)
    in_maps = []
    tok_ids = []
    for c in range(N_CORES):
        xt_c = np.zeros((EPC, H, pio), dtype=np.float32)
        toks = []
        for le in range(EPC):
            e = c * EPC + le
            te = order[starts[e]:starts[e + 1]]
            toks.append(te)
            xt_c[le, :, :len(te)] = xf[te].T
        tok_ids.append(toks)
        in_maps.append({
            "w": np.ascontiguousarray(wr[c]),
            "wda": np.ascontiguousarray(wda[c]),
            "wdb": np.ascontiguousarray(wdb[c]),
            "xt": xt_c.astype(bf16).reshape(EPC, HT, 128, pio)
                  .transpose(0, 2, 1, 3).copy(),
        })

    nc = _get_nc(pio)
    res = run_bass_kernel_spmd(nc, in_maps, core_ids=list(range(N_CORES)))

    out = np.empty((n, h), dtype=np.float32)
    for c in range(N_CORES):
        o = res.results[c]["out"]                # [EPC, 128, HT, pio] bf16
        for le in range(EPC):
            te = tok_ids[c][le]
            oe = np.asarray(o[le]).astype(np.float32) \
                .transpose(1, 0, 2).reshape(h, pio)      # [H, pio]
            out[te] = oe[:, :len(te)].T
    return out.reshape(b, s, h)


def kernel(x, expert_indices, gate_proj, up_proj, down_proj):
    """Full-input -> full-output entry point.

    Tokens-per-expert above _ROUND_CAP (pathological skew; PSUM bound)
    are handled by running the device kernel in multiple rounds over
    disjoint token slices - outputs are per-token independent."""
    idx = np.asarray(expert_indices)
    counts = np.bincount(idx.reshape(-1).astype(np.int64), minlength=E)
    if counts.max() <= _ROUND_CAP:
        return _kernel_once(x, expert_indices, gate_proj, up_proj, down_proj)

    b, s, h = x.shape
    n = b * s
    xf = np.ascontiguousarray(x, dtype=np.float32).reshape(n, h)
    idxf = idx.reshape(n).astype(np.int64)
    order = np.argsort(idxf, kind="stable")
    starts = np.zeros(E + 1, dtype=np.int64)
    np.cumsum(np.bincount(idxf, minlength=E), out=starts[1:])
    out = np.empty((n, h), dtype=np.float32)
    rounds = math.ceil(counts.max() / _ROUND_CAP)
    for r in range(rounds):
        sel = np.concatenate([
            order[starts[e] + r * _ROUND_CAP:
                  min(starts[e] + (r + 1) * _ROUND_CAP, starts[e + 1])]
            for e in range(E)])
        if not len(sel):
            continue
        xr = xf[sel].reshape(1, len(sel), h)
        ir = idxf[sel].reshape(1, len(sel))
        out[sel] = _kernel_once(
            xr, ir, gate_proj, up_proj, down_proj).reshape(len(sel), h)
    return out.reshape(b, s, h)


# revision 6
# speedup vs baseline: 1.0117x; 1.0117x over previous
"""Expert-parallel MoE MLP kernel for Trainium2 (8 NeuronCores).

Problem: x[B=2,S=1024,H=1024] f32, expert_indices[B,S] int, 16 experts,
gate/up_proj[E,H,I], down_proj[E,I,H] (H=I=1024):
    out[n] = silu(x_n @ Wg[e_n]) * (x_n @ Wu[e_n]) @ Wd[e_n].T

Sharding: expert parallelism - core c owns experts {2c, 2c+1}. The host
groups tokens by expert (the "all-to-all dispatch" runs on host since the
kernel contract is full-input -> full-output), pads each expert's token
block to a 16-multiple capacity, and each core runs dense per-expert GEMMs.

All operands are bf16 (rel err ~4e-3 vs the 2e-2 gate), which halves the
mandatory weight traffic to 12 MB/core - the roofline. Perf structure:
  - weight DMAs ride the SP HWDGE queue alone, in stream order, first in
    program order; nothing compute-dependent ever enters that FIFO, so
    the stream never stalls
  - xt and out DMAs ride the Activation HWDGE queue in parallel
  - matmuls run at the real (padded-to-16) token width: bf16 is
    1 cycle/row at any width, unlike f32r which needs >=256
  - gate/up accumulate h-outer into 8 PSUM banks so matmuls chase the
    arriving weight chunks; down_proj is packed in output-column chunks
    (3 double + 2 single j-tiles) so the last weight chunk feeds only 8
    short matmuls + one 37 KB store; that last store goes on the (by
    then idle) SP queue so it doesn't serialize behind the previous out
"""

import math

import numpy as np

E = 16
H = 1024
HT = 8           # H / 128 partition tiles
N_CORES = 8
EPC = E // N_CORES   # experts per core
NS = 4           # gate/up DMA chunks per projection (0.5 MB each)
HH = HT // NS    # h-tiles per gate/up chunk
# down chunks: j-tile groups per chunk (3 doubles, then 2 singles)
DCH = [(0, 2), (2, 2), (4, 2), (6, 1), (7, 1)]

_NC_CACHE = {}


def _build_nc(pio: int):
    """One SPMD program: EPC experts, pio real (DMA'd) token columns per
    expert. pio must be a multiple of 16, <= 512 (PSUM bank = 2 KB f32)."""
    import concourse.tile as tile
    from concourse import bacc, mybir
    from concourse.bass import ts

    f32 = mybir.dt.float32
    bf16 = mybir.dt.bfloat16

    nc = bacc.Bacc("TRN2", target_bir_lowering=False, debug=False,
                   num_devices=N_CORES)
    # gate/up packed partition-major: w[e, proj, p, h, :] = Wproj[e][h*128+p, :]
    w = nc.dram_tensor("w", [EPC, 2, 128, HT, H], bf16, kind="ExternalInput")
    # down packed j-sliced: wda[e, p, q, k, jj] = Wd[e].T[k*128+p, q*256+jj],
    # wdb[e, p, q, k, jj] = Wd[e].T[k*128+p, 768 + q*128+jj]
    wda = nc.dram_tensor("wda", [EPC, 128, 3, HT, 256], bf16,
                         kind="ExternalInput")
    wdb = nc.dram_tensor("wdb", [EPC, 128, 2, HT, 128], bf16,
                         kind="ExternalInput")
    xt = nc.dram_tensor("xt", [EPC, 128, HT, pio], bf16, kind="ExternalInput")
    out = nc.dram_tensor("out", [EPC, 128, HT, pio], bf16,
                         kind="ExternalOutput")

    with tile.TileContext(nc) as tc:
        with (
            tc.tile_pool(name="wp", bufs=13 * EPC) as wp,
            tc.tile_pool(name="xp", bufs=1) as xp,
            tc.tile_pool(name="gp", bufs=2) as gp,
            tc.tile_pool(name="ip", bufs=2) as ip,
            tc.tile_pool(name="op", bufs=2) as op,
            tc.tile_pool(name="ps", bufs=8, space="PSUM") as ps,
        ):
            # tokens for both experts, on the Act queue (parallel to weights)
            x_sb = xp.tile([128, EPC, HT, pio], bf16)
            for e in range(EPC):
                nc.scalar.dma_start(x_sb[:, e], xt[e])

            # the whole weight stream, enqueued up front in use order; every
            # tile is a distinct buffer so the FIFO never waits on compute
            wts = {}
            for e in range(EPC):
                for proj in range(2):
                    for q in range(NS):
                        t = wp.tile([128, HH, H], bf16, tag="w",
                                    name=f"w{e}{proj}{q}")
                        nc.sync.dma_start(t[:], w[e, proj, :, ts(q, HH), :])
                        wts[e, proj, q] = t
                for q, (j0, nj) in enumerate(DCH):
                    t = wp.tile([128, HT, nj * 128], bf16, tag="w",
                                name=f"wd{e}{q}")
                    if nj == 2:
                        nc.sync.dma_start(t[:], wda[e, :, q])
                    else:
                        nc.sync.dma_start(t[:], wdb[e, :, q - 3])
                    wts[e, 2, q] = t

            for e in range(EPC):
                g_sb = gp.tile([128, HT, pio], f32)
                i_sb = ip.tile([128, HT, pio], bf16)
                o_sb = op.tile([128, HT, pio], bf16, tag="o")
                # gate: h-outer accumulation into 8 PSUM banks
                g_ps = [ps.tile([128, pio], f32, tag="ps", name=f"g{e}{i}")
                        for i in range(HT)]
                for h in range(HT):
                    wt = wts[e, 0, h // HH]
                    for i in range(HT):
                        nc.tensor.matmul(
                            g_ps[i][:], wt[:, h % HH, ts(i, 128)],
                            x_sb[:, e, h], start=(h == 0), stop=(h == HT - 1))
                for i in range(HT):
                    nc.scalar.activation(
                        g_sb[:, i], g_ps[i][:],
                        mybir.ActivationFunctionType.Silu)
                # up
                u_ps = [ps.tile([128, pio], f32, tag="ps", name=f"u{e}{i}")
                        for i in range(HT)]
                for h in range(HT):
                    wt = wts[e, 1, h // HH]
                    for i in range(HT):
                        nc.tensor.matmul(
                            u_ps[i][:], wt[:, h % HH, ts(i, 128)],
                            x_sb[:, e, h], start=(h == 0), stop=(h == HT - 1))
                for i in range(HT):
                    nc.vector.tensor_mul(i_sb[:, i], g_sb[:, i], u_ps[i][:])
                # down: j-sliced chunks; each chunk finishes its j-tiles and
                # ships them immediately
                for q, (j0, nj) in enumerate(DCH):
                    wt = wts[e, 2, q]
                    for jl in range(nj):
                        o_ps = ps.tile([128, pio], f32, tag="ps",
                                       name=f"o{e}{q}{jl}")
                        for k in range(HT):
                            nc.tensor.matmul(
                                o_ps[:], wt[:, k, ts(jl, 128)], i_sb[:, k],
                                start=(k == 0), stop=(k == HT - 1))
                        nc.vector.tensor_copy(o_sb[:, j0 + jl], o_ps[:])
                    nc.scalar.dma_start(out[e, :, j0:j0 + nj, :],
                                        o_sb[:, j0:j0 + nj, :])
    nc.compile()
    return nc


def _get_nc(pio: int):
    if pio not in _NC_CACHE:
        _NC_CACHE[pio] = _build_nc(pio)
    return _NC_CACHE[pio]


_ROUND_CAP = 512          # max tokens/expert per device round (PSUM bank)


def _kernel_once(x, expert_indices, gate_proj, up_proj, down_proj):
    import ml_dtypes
    from concourse.bass_utils import run_bass_kernel_spmd

    bf16 = np.dtype(ml_dtypes.bfloat16)
    x = np.ascontiguousarray(x, dtype=np.float32)
    b, s, h = x.shape
    assert (h, gate_proj.shape) == (H, (E, H, H)), (x.shape, gate_proj.shape)

    n = b * s
    xf = x.reshape(n, h)
    idx = np.asarray(expert_indices).reshape(n).astype(np.int64)

    order = np.argsort(idx, kind="stable")       # token ids grouped by expert
    counts = np.bincount(idx, minlength=E)
    starts = np.zeros(E + 1, dtype=np.int64)
    np.cumsum(counts, out=starts[1:])
    maxc = int(counts.max())
    assert maxc <= _ROUND_CAP
    pio = max(16, 16 * math.ceil(maxc / 16))

    # per-core weight packing (bf16, partition-major)
    wr = np.stack([gate_proj, up_proj], axis=1).astype(bf16) \
        .reshape(N_CORES, EPC, 2, HT, 128, H).transpose(0, 1, 2, 4, 3, 5)
    dt = np.ascontiguousarray(down_proj.transpose(0, 2, 1)).astype(bf16)
    wda = dt[:, :, :768].reshape(N_CORES, EPC, HT, 128, 3, 256) \
        .transpose(0, 1, 3, 4, 2, 5)
    wdb = dt[:, :, 768:].reshape(N_CORES, EPC, HT, 128, 2, 128) \
        .transpose(0, 1, 3, 4, 2, 5)
    in_maps = []
    tok_ids = []
    for c in range(N_CORES):
        xt_c = np.zeros((EPC, H, pio), dtype=np.float32)
        toks = []
        for le in range(EPC):
            e = c * EPC + le
            te = order[starts[e]:starts[e + 1]]
            toks.append(te)
            xt_c[le, :, :len(te)] = xf[te].T
        tok_ids.append(toks)
        in_maps.append({
            "w": np.ascontiguousarray(wr[c]),
            "wda": np.ascontiguousarray(wda[c]),
            "wdb": np.ascontiguousarray(wdb[c]),
            "xt": xt_c.astype(bf16).reshape(EPC, HT, 128, pio)
                  .transpose(0, 2, 1, 3).copy(),
        })

    nc = _get_nc(pio)
    res = run_bass_kernel_spmd(nc, in_maps, core_ids=list(range(N_CORES)))

    out = np.empty((n, h), dtype=np.float32)
    for c in range(N_CORES):
        o = res.results[c]["out"]                # [EPC, 128, HT, pio] bf16
        for le in range(EPC):
            te = tok_ids[c][le]
            oe = np.asarray(o[le]).astype(np.float32) \
                .transpose(1, 0, 2).reshape(h, pio)      # [H, pio]
            out[te] = oe[:, :len(te)].T
    return out.reshape(b, s, h)


def kernel(x, expert_indices, gate_proj, up_proj, down_proj):
    """Full-input -> full-output entry point.

    Tokens-per-expert above _ROUND_CAP (pathological skew; PSUM bound)
    are handled by running the device kernel in multiple rounds over
    disjoint token slices - outputs are per-token independent."""
    idx = np.asarray(expert_indices)
    counts = np.bincount(idx.reshape(-1).astype(np.int64), minlength=E)
    if counts.max() <= _ROUND_CAP:
        return _kernel_once(x, expert_indices, gate_proj, up_proj, down_proj)

    b, s, h = x.shape
    n = b * s
    xf = np.ascontiguousarray(x, dtype=np.float32).reshape(n, h)
    idxf = idx.reshape(n).astype(np.int64)
    order = np.argsort(idxf, kind="stable")
    starts = np.zeros(E + 1, dtype=np.int64)
    np.cumsum(np.bincount(idxf, minlength=E), out=starts[1:])
    out = np.empty((n, h), dtype=np.float32)
    rounds = math.ceil(counts.max() / _ROUND_CAP)
    for r in range(rounds):
        sel = np.concatenate([
            order[starts[e] + r * _ROUND_CAP:
                  min(starts[e] + (r + 1) * _ROUND_CAP, starts[e + 1])]
            for e in range(E)])
        if not len(sel):
            continue
        xr = xf[sel].reshape(1, len(sel), h)
        ir = idxf[sel].reshape(1, len(sel))
        out[sel] = _kernel_once(
            xr, ir, gate_proj, up_proj, down_proj).reshape(len(sel), h)
    return out.reshape(b, s, h)


# revision 8
# speedup vs baseline: 1.0673x; 1.0550x over previous
"""Expert-parallel MoE MLP kernel for Trainium2 (8 NeuronCores).

Problem: x[B=2,S=1024,H=1024] f32, expert_indices[B,S] int, 16 experts,
gate/up_proj[E,H,I], down_proj[E,I,H] (H=I=1024):
    out[n] = silu(x_n @ Wg[e_n]) * (x_n @ Wu[e_n]) @ Wd[e_n].T

Sharding: expert parallelism - core c owns experts {2c, 2c+1}. The host
groups tokens by expert (the "all-to-all dispatch" runs on host since the
kernel contract is full-input -> full-output), pads each expert's token
block to a 16-multiple capacity, and each core runs dense per-expert GEMMs.

All operands are bf16 (rel err ~4e-3 vs the 2e-2 gate), which halves the
mandatory weight traffic to 12 MB/core - the roofline. Perf structure:
  - weight DMAs ride the SP HWDGE queue alone, in stream order; nothing
    compute-dependent ever enters that FIFO, so the stream never stalls.
    Keeping the queue small (17 DMAs) avoids a mid-stream sequencer
    throttle observed at ~26 queued DMAs.
  - xt and out DMAs ride the Activation HWDGE queue in parallel
  - matmuls run at the real (padded-to-16) token width: bf16 is
    1 cycle/row at any width, unlike f32r which needs >=256
  - expert 0's weights come as whole-projection chunks (compute runs
    mid-stream); expert 1 - whose compute defines the kernel tail - is
    chunked 0.5 MB for gate/up and j-sliced for down, ending in two
    single-j-tile chunks so the last weight byte feeds only 8 short
    matmuls + one 37 KB store
"""

import math

import numpy as np

E = 16
H = 1024
HT = 8           # H / 128 partition tiles
N_CORES = 8
EPC = E // N_CORES   # experts per core
NS = 4           # fine gate/up DMA chunks per projection (0.5 MB each)
HH = HT // NS    # h-tiles per fine gate/up chunk
# down j-tile chunks for the last expert (3 doubles, then 2 singles)
DCH = [(0, 2), (2, 2), (4, 2), (6, 1), (7, 1)]

_NC_CACHE = {}


def _build_nc(pio: int):
    """One SPMD program: EPC experts, pio real (DMA'd) token columns per
    expert. pio must be a multiple of 16, <= 512 (PSUM bank = 2 KB f32)."""
    import concourse.tile as tile
    from concourse import bacc, mybir
    from concourse.bass import ts

    f32 = mybir.dt.float32
    bf16 = mybir.dt.bfloat16

    nc = bacc.Bacc("TRN2", target_bir_lowering=False, debug=False,
                   num_devices=N_CORES)
    # gate/up packed partition-major: w[e, proj, p, h, :] = Wproj[e][h*128+p, :]
    w = nc.dram_tensor("w", [EPC, 2, 128, HT, H], bf16, kind="ExternalInput")
    # down packed j-sliced: wda[e, p, q, k, jj] = Wd[e].T[k*128+p, q*256+jj],
    # wdb[e, p, q, k, jj] = Wd[e].T[k*128+p, 768 + q*128+jj]
    wda = nc.dram_tensor("wda", [EPC, 128, 3, HT, 256], bf16,
                         kind="ExternalInput")
    wdb = nc.dram_tensor("wdb", [EPC, 128, 2, HT, 128], bf16,
                         kind="ExternalInput")
    xt = nc.dram_tensor("xt", [EPC, 128, HT, pio], bf16, kind="ExternalInput")
    out = nc.dram_tensor("out", [EPC, 128, HT, pio], bf16,
                         kind="ExternalOutput")

    with tile.TileContext(nc) as tc:
        with (
            tc.tile_pool(name="wb", bufs=4) as wb,
            tc.tile_pool(name="wp", bufs=13) as wp,
            tc.tile_pool(name="xp", bufs=1) as xp,
            tc.tile_pool(name="gp", bufs=2) as gp,
            tc.tile_pool(name="ip", bufs=2) as ip,
            tc.tile_pool(name="op", bufs=2) as op,
            tc.tile_pool(name="ps", bufs=8, space="PSUM") as ps,
        ):
            # tokens for both experts, on the Act queue (parallel to weights)
            x_sb = xp.tile([128, EPC, HT, pio], bf16)
            for e in range(EPC):
                nc.scalar.dma_start(x_sb[:, e], xt[e])

            # the whole weight stream, enqueued up front in use order; every
            # tile is a distinct buffer so the FIFO never waits on compute.
            # gu[e][proj][h] -> [128, H] stationary row block for h-tile h;
            # dn[e][jt] -> ([128, HT, *] tile, column offset) for j-tile jt.
            gu = [[None] * 2 for _ in range(EPC)]
            dn = [[None] * HT for _ in range(EPC)]

            # expert 0: whole-projection chunks (2 MB each, 16 KB runs)
            tg = wb.tile([128, HT, H], bf16, tag="wb", name="w0g")
            nc.sync.dma_start(tg[:], w[0, 0])
            gu[0][0] = [tg[:, h] for h in range(HT)]
            tu = wb.tile([128, HT, H], bf16, tag="wb", name="w0u")
            nc.sync.dma_start(tu[:], w[0, 1])
            gu[0][1] = [tu[:, h] for h in range(HT)]
            td = wb.tile([128, 3, HT, 256], bf16, tag="wb", name="w0d")
            nc.sync.dma_start(td[:], wda[0])
            for jt in range(6):
                dn[0][jt] = (td[:, jt // 2], (jt % 2) * 128)
            td2 = wb.tile([128, 2, HT, 128], bf16, tag="wb", name="w0d2")
            nc.sync.dma_start(td2[:], wdb[0])
            dn[0][6] = (td2[:, 0], 0)
            dn[0][7] = (td2[:, 1], 0)

            # expert 1: fine chunks - its compute defines the kernel tail
            for proj in range(2):
                rows = []
                for q in range(NS):
                    t = wp.tile([128, HH, H], bf16, tag="w",
                                name=f"w1{proj}{q}")
                    nc.sync.dma_start(t[:], w[1, proj, :, ts(q, HH), :])
                    rows += [t[:, hh] for hh in range(HH)]
                gu[1][proj] = rows
            for q, (j0, nj) in enumerate(DCH):
                t = wp.tile([128, HT, nj * 128], bf16, tag="w", name=f"w1d{q}")
                if nj == 2:
                    nc.sync.dma_start(t[:], wda[1, :, q])
                else:
                    nc.sync.dma_start(t[:], wdb[1, :, q - 3])
                for jl in range(nj):
                    dn[1][j0 + jl] = (t, jl * 128)

            for e in range(EPC):
                g_sb = gp.tile([128, HT, pio], f32)
                i_sb = ip.tile([128, HT, pio], bf16)
                o_sb = op.tile([128, HT, pio], bf16, tag="o")
                # gate: h-outer accumulation into 8 PSUM banks
                g_ps = [ps.tile([128, pio], f32, tag="ps", name=f"g{e}{i}")
                        for i in range(HT)]
                for h in range(HT):
                    for i in range(HT):
                        nc.tensor.matmul(
                            g_ps[i][:], gu[e][0][h][:, ts(i, 128)],
                            x_sb[:, e, h], start=(h == 0), stop=(h == HT - 1))
                for i in range(HT):
                    nc.scalar.activation(
                        g_sb[:, i], g_ps[i][:],
                        mybir.ActivationFunctionType.Silu)
                # up
                u_ps = [ps.tile([128, pio], f32, tag="ps", name=f"u{e}{i}")
                        for i in range(HT)]
                for h in range(HT):
                    for i in range(HT):
                        nc.tensor.matmul(
                            u_ps[i][:], gu[e][1][h][:, ts(i, 128)],
                            x_sb[:, e, h], start=(h == 0), stop=(h == HT - 1))
                for i in range(HT):
                    nc.vector.tensor_mul(i_sb[:, i], g_sb[:, i], u_ps[i][:])
                # down: per j-tile accumulation over k; expert 0 ships whole,
                # expert 1 ships per weight chunk (singles at the very end)
                for q, (j0, nj) in enumerate(DCH):
                    for jl in range(nj):
                        jt = j0 + jl
                        wt, col = dn[e][jt]
                        o_ps = ps.tile([128, pio], f32, tag="ps",
                                       name=f"o{e}{jt}")
                        for k in range(HT):
                            nc.tensor.matmul(
                                o_ps[:], wt[:, k, col:col + 128], i_sb[:, k],
                                start=(k == 0), stop=(k == HT - 1))
                        nc.vector.tensor_copy(o_sb[:, jt], o_ps[:])
                    if e > 0:
                        nc.scalar.dma_start(out[e, :, j0:j0 + nj, :],
                                            o_sb[:, j0:j0 + nj, :])
                if e == 0:
                    nc.scalar.dma_start(out[0], o_sb[:])
    nc.compile()
    return nc


def _get_nc(pio: int):
    if pio not in _NC_CACHE:
        _NC_CACHE[pio] = _build_nc(pio)
    return _NC_CACHE[pio]


_ROUND_CAP = 512          # max tokens/expert per device round (PSUM bank)


def _kernel_once(x, expert_indices, gate_proj, up_proj, down_proj):
    import ml_dtypes
    from concourse.bass_utils import run_bass_kernel_spmd

    bf16 = np.dtype(ml_dtypes.bfloat16)
    x = np.ascontiguousarray(x, dtype=np.float32)
    b, s, h = x.shape
    assert (h, gate_proj.shape) == (H, (E, H, H)), (x.shape, gate_proj.shape)

    n = b * s
    xf = x.reshape(n, h)
    idx = np.asarray(expert_indices).reshape(n).astype(np.int64)

    order = np.argsort(idx, kind="stable")       # token ids grouped by expert
    counts = np.bincount(idx, minlength=E)
    starts = np.zeros(E + 1, dtype=np.int64)
    np.cumsum(counts, out=starts[1:])
    maxc = int(counts.max())
    assert maxc <= _ROUND_CAP
    pio = max(16, 16 * math.ceil(maxc / 16))

    # per-core weight packing (bf16, partition-major)
    wr = np.stack([gate_proj, up_proj], axis=1).astype(bf16) \
        .reshape(N_CORES, EPC, 2, HT, 128, H).transpose(0, 1, 2, 4, 3, 5)
    dt = np.ascontiguousarray(down_proj.transpose(0, 2, 1)).astype(bf16)
    wda = dt[:, :, :768].reshape(N_CORES, EPC, HT, 128, 3, 256) \
        .transpose(0, 1, 3, 4, 2, 5)
    wdb = dt[:, :, 768:].reshape(N_CORES, EPC, HT, 128, 2, 128) \
        .transpose(0, 1, 3, 4, 2, 5)
    in_maps = []
    tok_ids = []
    for c in range(N_CORES):
        xt_c = np.zeros((EPC, H, pio), dtype=np.float32)
        toks = []
        for le in range(EPC):
            e = c * EPC + le
            te = order[starts[e]:starts[e + 1]]
            toks.append(te)
            xt_c[le, :, :len(te)] = xf[te].T
        tok_ids.append(toks)
        in_maps.append({
            "w": np.ascontiguousarray(wr[c]),
            "wda": np.ascontiguousarray(wda[c]),
            "wdb": np.ascontiguousarray(wdb[c]),
            "xt": xt_c.astype(bf16).reshape(EPC, HT, 128, pio)
                  .transpose(0, 2, 1, 3).copy(),
        })

    nc = _get_nc(pio)
    res = run_bass_kernel_spmd(nc, in_maps, core_ids=list(range(N_CORES)))

    out = np.empty((n, h), dtype=np.float32)
    for c in range(N_CORES):
        o = res.results[c]["out"]                # [EPC, 128, HT, pio] bf16
        for le in range(EPC):
            te = tok_ids[c][le]
            oe = np.asarray(o[le]).astype(np.float32) \
                .transpose(1, 0, 2).reshape(h, pio)      # [H, pio]
            out[te] = oe[:, :len(te)].T
    return out.reshape(b, s, h)


def kernel(x, expert_indices, gate_proj, up_proj, down_proj):
    """Full-input -> full-output entry point.

    Tokens-per-expert above _ROUND_CAP (pathological skew; PSUM bound)
    are handled by running the device kernel in multiple rounds over
    disjoint token slices - outputs are per-token independent."""
    idx = np.asarray(expert_indices)
    counts = np.bincount(idx.reshape(-1).astype(np.int64), minlength=E)
    if counts.max() <= _ROUND_CAP:
        return _kernel_once(x, expert_indices, gate_proj, up_proj, down_proj)

    b, s, h = x.shape
    n = b * s
    xf = np.ascontiguousarray(x, dtype=np.float32).reshape(n, h)
    idxf = idx.reshape(n).astype(np.int64)
    order = np.argsort(idxf, kind="stable")
    starts = np.zeros(E + 1, dtype=np.int64)
    np.cumsum(np.bincount(idxf, minlength=E), out=starts[1:])
    out = np.empty((n, h), dtype=np.float32)
    rounds = math.ceil(counts.max() / _ROUND_CAP)
    for r in range(rounds):
        sel = np.concatenate([
            order[starts[e] + r * _ROUND_CAP:
                  min(starts[e] + (r + 1) * _ROUND_CAP, starts[e + 1])]
            for e in range(E)])
        if not len(sel):
            continue
        xr = xf[sel].reshape(1, len(sel), h)
        ir = idxf[sel].reshape(1, len(sel))
        out[sel] = _kernel_once(
            xr, ir, gate_proj, up_proj, down_proj).reshape(len(sel), h)
    return out.reshape(b, s, h)


# revision 9
# speedup vs baseline: 1.1652x; 1.0917x over previous
"""Expert-parallel MoE MLP kernel for Trainium2 (8 NeuronCores).

Problem: x[B=2,S=1024,H=1024] f32, expert_indices[B,S] int, 16 experts,
gate/up_proj[E,H,I], down_proj[E,I,H] (H=I=1024):
    out[n] = silu(x_n @ Wg[e_n]) * (x_n @ Wu[e_n]) @ Wd[e_n].T

Sharding: expert parallelism - core c owns experts {2c, 2c+1}. The host
groups tokens by expert (the "all-to-all dispatch" runs on host since the
kernel contract is full-input -> full-output), pads each expert's token
block to a 16-multiple capacity, and each core runs dense per-expert GEMMs.

All operands are bf16 (rel err ~4e-3 vs the 2e-2 gate), which halves the
mandatory weight traffic to 12 MB/core - the roofline. Perf structure:
  - the weight stream rides the SP HWDGE queue in use order as uniform
    0.5 MB chunks (4 KB per-partition runs). HWDGE rotates only 8
    completion semaphores per queue, so DMA k+8 dispatches only after
    DMA k's consumers ran: chunks must be small and their matmuls must
    chase arrival closely, or the stream stalls (2 MB chunks and
    late-consumed chunks both measurably break streaming).
  - xt for expert 1 sits in the SP FIFO right before expert 1's
    weights - the slot where its consumers (expert 1's gate matmuls)
    run; xt for expert 0 and all out stores ride the Act HWDGE queue.
  - matmuls run at the real (padded-to-16) token width: bf16 is
    1 cycle/row at any width, unlike f32r which needs >=256.
  - gate/up accumulate h-outer into 8 PSUM banks chasing the stream;
    down_proj chunks are output-column slices so each chunk's results
    ship immediately; the final [j6,j7] chunk ships j6 and j7 as
    separate 37 KB stores so the kernel tail is just 8 matmuls + copy
    + store deep.
"""

import math

import numpy as np

E = 16
H = 1024
HT = 8           # H / 128 partition tiles
N_CORES = 8
EPC = E // N_CORES   # experts per core
NS = 4           # DMA chunks per projection (0.5 MB each)
HH = HT // NS    # h-tiles per gate/up chunk

_NC_CACHE = {}


def _build_nc(pio: int):
    """One SPMD program: EPC experts, pio real (DMA'd) token columns per
    expert. pio must be a multiple of 16, <= 512 (PSUM bank = 2 KB f32)."""
    import concourse.tile as tile
    from concourse import bacc, mybir
    from concourse.bass import ts

    f32 = mybir.dt.float32
    bf16 = mybir.dt.bfloat16

    nc = bacc.Bacc("TRN2", target_bir_lowering=False, debug=False,
                   num_devices=N_CORES)
    # gate/up packed partition-major: w[e, proj, p, h, :] = Wproj[e][h*128+p, :]
    w = nc.dram_tensor("w", [EPC, 2, 128, HT, H], bf16, kind="ExternalInput")
    # down packed j-sliced: wd[e, p, q, k, jj] = Wd[e].T[k*128+p, q*256+jj]
    wd = nc.dram_tensor("wd", [EPC, 128, NS, HT, 256], bf16,
                        kind="ExternalInput")
    xt = nc.dram_tensor("xt", [EPC, 128, HT, pio], bf16, kind="ExternalInput")
    out = nc.dram_tensor("out", [EPC, 128, HT, pio], bf16,
                         kind="ExternalOutput")

    with tile.TileContext(nc) as tc:
        with (
            tc.tile_pool(name="wp", bufs=12 * EPC) as wp,
            tc.tile_pool(name="xp", bufs=1) as xp,
            tc.tile_pool(name="gp", bufs=2) as gp,
            tc.tile_pool(name="ip", bufs=2) as ip,
            tc.tile_pool(name="op", bufs=2) as op,
            tc.tile_pool(name="ps", bufs=8, space="PSUM") as ps,
        ):
            x_sb = xp.tile([128, EPC, HT, pio], bf16)
            # xt for expert 0 on the Act queue (parallel to the weight FIFO)
            nc.scalar.dma_start(x_sb[:, 0], xt[0])

            # weight stream on the SP queue, in use order; xt for expert 1
            # rides in the FIFO slot where its consumers run
            wts = {}
            for e in range(EPC):
                if e > 0:
                    nc.sync.dma_start(x_sb[:, e], xt[e])
                for proj in range(2):
                    for q in range(NS):
                        t = wp.tile([128, HH, H], bf16, tag="w",
                                    name=f"w{e}{proj}{q}")
                        nc.sync.dma_start(t[:], w[e, proj, :, ts(q, HH), :])
                        wts[e, proj, q] = t
                for q in range(NS):
                    t = wp.tile([128, HT, 256], bf16, tag="w",
                                name=f"wd{e}{q}")
                    nc.sync.dma_start(t[:], wd[e, :, q])
                    wts[e, 2, q] = t

            for e in range(EPC):
                g_sb = gp.tile([128, HT, pio], f32)
                i_sb = ip.tile([128, HT, pio], bf16)
                o_sb = op.tile([128, HT, pio], bf16, tag="o")
                # gate: h-outer accumulation into 8 PSUM banks
                g_ps = [ps.tile([128, pio], f32, tag="ps", name=f"g{e}{i}")
                        for i in range(HT)]
                for h in range(HT):
                    wt = wts[e, 0, h // HH]
                    for i in range(HT):
                        nc.tensor.matmul(
                            g_ps[i][:], wt[:, h % HH, ts(i, 128)],
                            x_sb[:, e, h], start=(h == 0), stop=(h == HT - 1))
                for i in range(HT):
                    nc.scalar.activation(
                        g_sb[:, i], g_ps[i][:],
                        mybir.ActivationFunctionType.Silu)
                # up
                u_ps = [ps.tile([128, pio], f32, tag="ps", name=f"u{e}{i}")
                        for i in range(HT)]
                for h in range(HT):
                    wt = wts[e, 1, h // HH]
                    for i in range(HT):
                        nc.tensor.matmul(
                            u_ps[i][:], wt[:, h % HH, ts(i, 128)],
                            x_sb[:, e, h], start=(h == 0), stop=(h == HT - 1))
                for i in range(HT):
                    nc.vector.tensor_mul(i_sb[:, i], g_sb[:, i], u_ps[i][:])
                # down: j-sliced chunks; each chunk finishes its 2 j-tiles
                # and ships them. The very last chunk ships its j-tiles as
                # two separate stores so the tail is one j-tile deep.
                for q in range(NS):
                    wt = wts[e, 2, q]
                    split = e == EPC - 1 and q == NS - 1
                    for jl in range(2):
                        jt = 2 * q + jl
                        o_ps = ps.tile([128, pio], f32, tag="ps",
                                       name=f"o{e}{jt}")
                        for k in range(HT):
                            nc.tensor.matmul(
                                o_ps[:], wt[:, k, ts(jl, 128)], i_sb[:, k],
                                start=(k == 0), stop=(k == HT - 1))
                        nc.vector.tensor_copy(o_sb[:, jt], o_ps[:])
                        if split:
                            nc.scalar.dma_start(out[e, :, jt:jt + 1, :],
                                                o_sb[:, jt:jt + 1])
                    if not split:
                        nc.scalar.dma_start(out[e, :, 2 * q:2 * q + 2, :],
                                            o_sb[:, 2 * q:2 * q + 2])
    nc.compile()
    return nc


def _get_nc(pio: int):
    if pio not in _NC_CACHE:
        _NC_CACHE[pio] = _build_nc(pio)
    return _NC_CACHE[pio]


_ROUND_CAP = 512          # max tokens/expert per device round (PSUM bank)


def _kernel_once(x, expert_indices, gate_proj, up_proj, down_proj):
    import ml_dtypes
    from concourse.bass_utils import run_bass_kernel_spmd

    bf16 = np.dtype(ml_dtypes.bfloat16)
    x = np.ascontiguousarray(x, dtype=np.float32)
    b, s, h = x.shape
    assert (h, gate_proj.shape) == (H, (E, H, H)), (x.shape, gate_proj.shape)

    n = b * s
    xf = x.reshape(n, h)
    idx = np.asarray(expert_indices).reshape(n).astype(np.int64)

    order = np.argsort(idx, kind="stable")       # token ids grouped by expert
    counts = np.bincount(idx, minlength=E)
    starts = np.zeros(E + 1, dtype=np.int64)
    np.cumsum(counts, out=starts[1:])
    maxc = int(counts.max())
    assert maxc <= _ROUND_CAP
    pio = max(16, 16 * math.ceil(maxc / 16))

    # per-core weight packing (bf16, partition-major)
    wr = np.stack([gate_proj, up_proj], axis=1).astype(bf16) \
        .reshape(N_CORES, EPC, 2, HT, 128, H).transpose(0, 1, 2, 4, 3, 5)
    wdr = np.ascontiguousarray(down_proj.transpose(0, 2, 1)).astype(bf16) \
        .reshape(N_CORES, EPC, HT, 128, NS, 256).transpose(0, 1, 3, 4, 2, 5)
    in_maps = []
    tok_ids = []
    for c in range(N_CORES):
        xt_c = np.zeros((EPC, H, pio), dtype=np.float32)
        toks = []
        for le in range(EPC):
            e = c * EPC + le
            te = order[starts[e]:starts[e + 1]]
            toks.append(te)
            xt_c[le, :, :len(te)] = xf[te].T
        tok_ids.append(toks)
        in_maps.append({
            "w": np.ascontiguousarray(wr[c]),
            "wd": np.ascontiguousarray(wdr[c]),
            "xt": xt_c.astype(bf16).reshape(EPC, HT, 128, pio)
                  .transpose(0, 2, 1, 3).copy(),
        })

    nc = _get_nc(pio)
    res = run_bass_kernel_spmd(nc, in_maps, core_ids=list(range(N_CORES)))

    out = np.empty((n, h), dtype=np.float32)
    for c in range(N_CORES):
        o = res.results[c]["out"]                # [EPC, 128, HT, pio] bf16
        for le in range(EPC):
            te = tok_ids[c][le]
            oe = np.asarray(o[le]).astype(np.float32) \
                .transpose(1, 0, 2).reshape(h, pio)      # [H, pio]
            out[te] = oe[:, :len(te)].T
    return out.reshape(b, s, h)


def kernel(x, expert_indices, gate_proj, up_proj, down_proj):
    """Full-input -> full-output entry point.

    Tokens-per-expert above _ROUND_CAP (pathological skew; PSUM bound)
    are handled by running the device kernel in multiple rounds over
    disjoint token slices - outputs are per-token independent."""
    idx = np.asarray(expert_indices)
    counts = np.bincount(idx.reshape(-1).astype(np.int64), minlength=E)
    if counts.max() <= _ROUND_CAP:
        return _kernel_once(x, expert_indices, gate_proj, up_proj, down_proj)

    b, s, h = x.shape
    n = b * s
    xf = np.ascontiguousarray(x, dtype=np.float32).reshape(n, h)
    idxf = idx.reshape(n).astype(np.int64)
    order = np.argsort(idxf, kind="stable")
    starts = np.zeros(E + 1, dtype=np.int64)
    np.cumsum(np.bincount(idxf, minlength=E), out=starts[1:])
    out = np.empty((n, h), dtype=np.float32)
    rounds = math.ceil(counts.max() / _ROUND_CAP)
    for r in range(rounds):
        sel = np.concatenate([
            order[starts[e] + r * _ROUND_CAP:
                  min(starts[e] + (r + 1) * _ROUND_CAP, starts[e + 1])]
            for e in range(E)])
        if not len(sel):
            continue
        xr = xf[sel].reshape(1, len(sel), h)
        ir = idxf[sel].reshape(1, len(sel))
        out[sel] = _kernel_once(
            xr, ir, gate_proj, up_proj, down_proj).reshape(len(sel), h)
    return out.reshape(b, s, h)
